# revision 1
# baseline (speedup 1.0000x reference)
"""Sliding-window MQA attention block on 8 Trainium2 NeuronCores.

Sharding: sequence-parallel. 8 cores = 2 batches x 4 query-chunks of 512
tokens. Each core loads its 512 query tokens plus a 256-token K/V halo
(768 KV tokens total, zero-padded in front for chunk 0), computes the
Q/K/V projections, windowed attention for all 16 heads, and the final
projection locally. No collectives; the host concatenates chunk outputs.

Device algorithm (per core), logits kept in [t, s] orientation:
  qT[1024, 512]  = WqT.T @ xqT        (per 128-row blocks)
  ktd[128, 768]  = K^T duplicated into both partition halves (MQA shared)
  v_aug[768, 65] = V with an all-ones column (gives softmax denominators)
  per head h, per 128-query block tb (s-window = 384 = 128 + 256 halo):
    logits[128, 384] = qh_tb.T @ kT[:, window]
    probs = exp(0.125 * logits) * band   (band = 0/1 sliding-window mask)
    probsT pieces via PE transpose; out[t, 65] = sum_sb probsT_sb.T @ v_aug
    attn[t, 64h:64h+64] = out[:, :64] * (1 / out[:, 64])
  attnT via PE transpose; final[512, 1024] = attnT.T @ WfT + bias
"""

import math
import os
import sys

import numpy as np

for _p in ("/opt/trn_rl_repo",):
    if _p not in sys.path and os.path.isdir(_p):
        sys.path.insert(0, _p)

import ml_dtypes

import concourse.bass as bass
import concourse.mybir as mybir
import concourse.tile as tile
from concourse import bacc
from concourse.bass_utils import run_bass_kernel_spmd
from concourse.masks import make_identity

WIDTH = 1024
H = 16
HD = 64
WIN = 256
T = 512          # query tokens per core
KV = 768         # kv tokens per core (256 halo + 512)
NKB = WIDTH // 128
NTB = T // 128
NSB = KV // 128
WINW = 384       # s-window per 128-query block
F32 = mybir.dt.float32

USE_BF16 = os.environ.get("KERNEL_F32", "0") != "1"
DT = mybir.dt.bfloat16 if USE_BF16 else mybir.dt.float32
NPDT = ml_dtypes.bfloat16 if USE_BF16 else np.float32


def build_kernel():
    nc = bacc.Bacc(None, target_bir_lowering=False)

    xkvT_d = nc.dram_tensor("xkvT", [WIDTH, KV], DT, kind="ExternalInput")
    wqT_d = nc.dram_tensor("wqT", [WIDTH, WIDTH], DT, kind="ExternalInput")
    wkT_d = nc.dram_tensor("wkT", [WIDTH, HD], DT, kind="ExternalInput")
    wvT_d = nc.dram_tensor("wvT", [WIDTH, HD], DT, kind="ExternalInput")
    wfT_d = nc.dram_tensor("wfT", [WIDTH, WIDTH], DT, kind="ExternalInput")
    band_d = nc.dram_tensor("band", [128, NTB, WINW], DT, kind="ExternalInput")
    bias_d = nc.dram_tensor("biasb", [128, WIDTH], F32, kind="ExternalInput")
    out_d = nc.dram_tensor("out", [T, WIDTH], F32, kind="ExternalOutput")

    with tile.TileContext(nc) as tc:
        with tc.tile_pool(name="persist", bufs=1) as pp:
            # ---- load inputs ----
            xkv_t = []
            for i in range(NKB):
                t_ = pp.tile([128, KV], DT, tag=f"xkv{i}", name=f"xkv{i}")
                nc.sync.dma_start(t_[:], xkvT_d[128 * i : 128 * (i + 1), :])
                xkv_t.append(t_)
            wq_t = []
            wf_t = []
            for i in range(NKB):
                t_ = pp.tile([128, WIDTH], DT, tag=f"wq{i}", name=f"wq{i}")
                nc.sync.dma_start(t_[:], wqT_d[128 * i : 128 * (i + 1), :])
                wq_t.append(t_)
                t_ = pp.tile([128, WIDTH], DT, tag=f"wf{i}", name=f"wf{i}")
                nc.sync.dma_start(t_[:], wfT_d[128 * i : 128 * (i + 1), :])
                wf_t.append(t_)
            wk_t = []
            wv_t = []
            for i in range(NKB):
                t_ = pp.tile([128, HD], DT, tag=f"wk{i}", name=f"wk{i}")
                nc.sync.dma_start(t_[:], wkT_d[128 * i : 128 * (i + 1), :])
                wk_t.append(t_)
                t_ = pp.tile([128, HD], DT, tag=f"wv{i}", name=f"wv{i}")
                nc.sync.dma_start(t_[:], wvT_d[128 * i : 128 * (i + 1), :])
                wv_t.append(t_)
            band_t = pp.tile([128, NTB, WINW], DT, tag="band")
            nc.sync.dma_start(band_t[:], band_d[:, :, :])
            bias_t = pp.tile([128, WIDTH], F32, tag="bias")
            nc.sync.dma_start(bias_t[:], bias_d[:, :])

            ident = pp.tile([128, 128], DT, tag="ident")
            make_identity(nc, ident[:])

            # ---- persistent intermediates ----
            qT_t = [pp.tile([128, T], DT, tag=f"qT{i}", name=f"qT{i}") for i in range(NKB)]
            ktd = pp.tile([128, KV], DT, tag="ktd")
            vaug = [pp.tile([128, HD + 1], DT, tag=f"vaug{i}", name=f"vaug{i}") for i in range(NSB)]
            attn_t = [pp.tile([128, WIDTH], DT, tag=f"attn{i}", name=f"attn{i}") for i in range(NTB)]
            attnT_t = [pp.tile([128, T], DT, tag=f"attnT{i}", name=f"attnT{i}") for i in range(NKB)]

            # ---- phase 1: projections ----
            with (
                tc.tile_pool(name="psq", bufs=2, space="PSUM") as psq_pool,
                tc.tile_pool(name="psk", bufs=1, space="PSUM") as psk_pool,
                tc.tile_pool(name="psv", bufs=2, space="PSUM") as psv_pool,
            ):
                for mb in range(NKB):
                    pq = psq_pool.tile([128, T], F32, tag="pq")
                    for kb in range(NKB):
                        nc.tensor.matmul(
                            pq[:],
                            lhsT=wq_t[kb][:, 128 * mb : 128 * (mb + 1)],
                            rhs=xkv_t[kb][:, WIN : WIN + T],
                            start=(kb == 0),
                            stop=(kb == NKB - 1),
                        )
                    nc.vector.tensor_copy(qT_t[mb][:], pq[:])

                pk = psk_pool.tile([128, KV], F32, tag="pk")
                for half in (0, 64):
                    for seg0, segw in ((0, 512), (512, 256)):
                        for kb in range(NKB):
                            nc.tensor.matmul(
                                pk[half : half + 64, seg0 : seg0 + segw],
                                lhsT=wk_t[kb][:],
                                rhs=xkv_t[kb][:, seg0 : seg0 + segw],
                                start=(kb == 0),
                                stop=(kb == NKB - 1),
                            )
                nc.vector.tensor_copy(ktd[:], pk[:])

                for sb in range(NSB):
                    pv = psv_pool.tile([128, HD], F32, tag="pv")
                    for kb in range(NKB):
                        nc.tensor.matmul(
                            pv[:],
                            lhsT=xkv_t[kb][:, 128 * sb : 128 * (sb + 1)],
                            rhs=wv_t[kb][:],
                            start=(kb == 0),
                            stop=(kb == NKB - 1),
                        )
                    nc.scalar.copy(vaug[sb][:, 0:HD], pv[:])
                    nc.gpsimd.memset(vaug[sb][:, HD : HD + 1], 1.0)

            # ---- phase 2: attention ----
            with (
                tc.tile_pool(name="psl", bufs=2, space="PSUM") as psl_pool,
                tc.tile_pool(name="pst", bufs=2, space="PSUM") as pst_pool,
                tc.tile_pool(name="pso", bufs=2, space="PSUM") as pso_pool,
                tc.tile_pool(name="awork", bufs=3) as awork,
            ):
                for h in range(H):
                    mb, half = divmod(h, 2)
                    hb = 64 * half
                    qh = qT_t[mb]
                    probs = awork.tile([128, NTB, WINW], DT, tag="probs")
                    for pair in range(2):
                        pl = psl_pool.tile([128, 2, 512], F32, tag="pl")
                        for u in range(2):
                            tb = 2 * pair + u
                            nc.tensor.matmul(
                                pl[:, u, 0:WINW],
                                lhsT=qh[hb : hb + 64, 128 * tb : 128 * (tb + 1)],
                                rhs=ktd[hb : hb + 64, 128 * tb : 128 * tb + WINW],
                                start=True,
                                stop=True,
                            )
                        nc.scalar.activation(
                            out=probs[:, 2 * pair : 2 * pair + 2, :],
                            in_=pl[:, :, 0:WINW],
                            func=mybir.ActivationFunctionType.Exp,
                            scale=0.125,
                        )
                    probsm = awork.tile([128, NTB, WINW], DT, tag="probsm")
                    nc.vector.tensor_mul(probsm[:], probs[:], band_t[:])

                    po = pso_pool.tile([128, NTB, 128], F32, tag="po")
                    for tb in range(NTB):
                        pt = pst_pool.tile([128, WINW], DT, tag="pt")
                        for k3 in range(3):
                            nc.tensor.transpose(
                                pt[:, 128 * k3 : 128 * (k3 + 1)],
                                probsm[:, tb, 128 * k3 : 128 * (k3 + 1)],
                                ident[:],
                            )
                        pT_sb = awork.tile([128, WINW], DT, tag="pTs")
                        nc.vector.tensor_copy(pT_sb[:], pt[:])
                        for k3 in range(3):
                            nc.tensor.matmul(
                                po[:, tb, 0 : HD + 1],
                                lhsT=pT_sb[:, 128 * k3 : 128 * (k3 + 1)],
                                rhs=vaug[tb + k3][:],
                                start=(k3 == 0),
                                stop=(k3 == 2),
                            )
                    recip = awork.tile([128, NTB, 1], F32, tag="recip")
                    nc.vector.reciprocal(recip[:], po[:, :, HD : HD + 1])
                    for tb in range(NTB):
                        nc.vector.tensor_scalar_mul(
                            attn_t[tb][:, 64 * h : 64 * (h + 1)],
                            po[:, tb, 0:HD],
                            recip[:, tb, :],
                        )

            # attn -> attnT for the final projection
            with (
                tc.tile_pool(name="psat", bufs=2, space="PSUM") as psat_pool,
            ):
                for wb in range(NKB):
                    pat = psat_pool.tile([128, NTB, 128], DT, tag="pat")
                    for tb in range(NTB):
                        nc.tensor.transpose(
                            pat[:, tb, :],
                            attn_t[tb][:, 128 * wb : 128 * (wb + 1)],
                            ident[:],
                        )
                    nc.vector.tensor_copy(attnT_t[wb][:], pat[:])

            # ---- phase 3: final projection + bias ----
            with (
                tc.tile_pool(name="psf", bufs=4, space="PSUM") as psf_pool,
                tc.tile_pool(name="fin", bufs=3) as fin_pool,
            ):
                for tb in range(NTB):
                    for nh in range(2):
                        pf = psf_pool.tile([128, 512], F32, tag="pf")
                        for wb in range(NKB):
                            nc.tensor.matmul(
                                pf[:],
                                lhsT=attnT_t[wb][:, 128 * tb : 128 * (tb + 1)],
                                rhs=wf_t[wb][:, 512 * nh : 512 * (nh + 1)],
                                start=(wb == 0),
                                stop=(wb == NKB - 1),
                            )
                        fo = fin_pool.tile([128, 512], F32, tag="fo")
                        nc.vector.tensor_add(
                            fo[:], pf[:], bias_t[:, 512 * nh : 512 * (nh + 1)]
                        )
                        nc.sync.dma_start(
                            out_d[128 * tb : 128 * (tb + 1), 512 * nh : 512 * (nh + 1)],
                            fo[:],
                        )

    return nc


def _prep_core_inputs(x, Wq, Wk, Wv, Wf, bf, core):
    bi, ch = divmod(core, 4)
    qs = T * ch
    ks = qs - WIN
    xkvT = np.zeros((WIDTH, KV), np.float32)
    lo = max(ks, 0)
    xkvT[:, lo - ks :] = x[bi, lo : qs + T, :].T

    band = np.zeros((128, NTB, WINW), np.float32)
    p = np.arange(128)[:, None]
    f = np.arange(WINW)[None, :]
    base = (f - p >= 0) & (f - p <= WIN)
    for tb in range(NTB):
        band[:, tb, :] = base & (ks + 128 * tb + f >= 0)

    return {
        "xkvT": np.ascontiguousarray(xkvT).astype(NPDT),
        "wqT": np.ascontiguousarray(Wq.T).astype(NPDT),
        "wkT": np.ascontiguousarray(Wk.T).astype(NPDT),
        "wvT": np.ascontiguousarray(Wv.T).astype(NPDT),
        "wfT": np.ascontiguousarray(Wf.T).astype(NPDT),
        "band": band.astype(NPDT),
        "biasb": np.ascontiguousarray(
            np.broadcast_to(bf.astype(np.float32), (128, WIDTH))
        ),
    }


_RUN_KW = {}  # test.py can inject trace=True etc.
_LAST_RESULT = [None]


def kernel(x, segment_pos, Wq, Wk, Wv, Wf, bf):
    x = np.asarray(x, np.float32)
    Wq = np.asarray(Wq, np.float32)
    Wk = np.asarray(Wk, np.float32)
    Wv = np.asarray(Wv, np.float32)
    Wf = np.asarray(Wf, np.float32)
    bf = np.asarray(bf, np.float32)

    nc = build_kernel()
    nc.finalize()
    in_maps = [_prep_core_inputs(x, Wq, Wk, Wv, Wf, bf, c) for c in range(8)]
    res = run_bass_kernel_spmd(nc, in_maps, core_ids=list(range(8)), **_RUN_KW)
    _LAST_RESULT[0] = res

    b, t = x.shape[0], x.shape[1]
    out = np.empty((b, t, WIDTH), np.float32)
    for c in range(8):
        bi, ch = divmod(c, 4)
        out[bi, T * ch : T * (ch + 1)] = res.results[c]["out"]
    return out



# revision 9
# speedup vs baseline: 1.0184x; 1.0184x over previous
"""Sliding-window MQA attention block on 8 Trainium2 NeuronCores.

Sharding: sequence-parallel. 8 cores = 2 batches x 4 query-chunks of 512
tokens. Each core loads its 512 query tokens plus a 256-token K/V halo
(768 KV tokens total, zero-padded in front for chunk 0), computes the
Q/K/V projections, windowed attention for all 16 heads, and the final
projection locally. No collectives; the host concatenates chunk outputs.

All inputs arrive in ONE packed bf16 DRAM tensor [128, PK_COLS] (three
dma_starts into three SBUF staging tiles so compute can start early).

Device algorithm (per core), logits computed TRANSPOSED ([s, t]) so no
PE transposes of probs are needed:
  qT[1024, 512]  = WqT.T @ xqT            (per 128-row blocks; [hd, t])
  ktd[128, 768]  = K^T duplicated into both partition halves (MQA shared)
  vaug[128,6,65] = V with a validity column (gives softmax denominators
                   AND zeroes out the padded kv positions of chunk 0)
  per head-pair, per 128-query block tb (window = 3 kv blocks, diagonal):
    logitsT[s,t] blocks via matmul(lhsT=ktd[hd, s-blk], rhs=qT[hd, t-blk])
    probsT = exp(0.125 * logitsT)         (one ACT op per pair x tb)
    two triangular 128x128 masks (k3=0 lower, k3=2 upper) in one strided
    vector multiply; middle diagonal needs no mask
    po[t, 65] += probsT_blk.T @ vaug_blk  (PE, accumulate 3 diagonals)
    attn[t, 64h:64h+64] = po[:, :64] * (1 / po[:, 64])
  attnT via PE transpose; final[512, 1024] = attnT.T @ WfT + bias
"""

import math
import os
import sys

import numpy as np

for _p in ("/opt/trn_rl_repo",):
    if _p not in sys.path and os.path.isdir(_p):
        sys.path.insert(0, _p)

import ml_dtypes

import concourse.bass as bass
import concourse.mybir as mybir
import concourse.tile as tile
from concourse import bacc
from concourse.bass_utils import run_bass_kernel_spmd
from concourse.masks import make_identity

WIDTH = 1024
H = 16
HD = 64
WIN = 256
T = 512          # query tokens per core
KV = 768         # kv tokens per core (256 halo + 512)
NKB = WIDTH // 128
NTB = T // 128
NSB = KV // 128
F32 = mybir.dt.float32
DT = mybir.dt.bfloat16
NPDT = ml_dtypes.bfloat16

# packed input column offsets (bf16 columns)
COL_XKV = 0                       # 8 x 768
COL_WK = COL_XKV + NKB * KV       # 8 x 64
COL_WV = COL_WK + NKB * HD        # 8 x 64
LEN_A = COL_WV + NKB * HD         # staging tile A ends here (7168)
COL_WQ = LEN_A                    # 8 x 1024
COL_TRI = COL_WQ + NKB * WIDTH    # [128, 512]: lo|hi|lo|hi
COL_VALID = COL_TRI + 512         # [128, 8] (6 used)
LEN_B = COL_VALID + 8 - LEN_A     # staging tile B cols (8712)
COL_WF = COL_VALID + 8            # 8 x 1024
COL_BIAS = COL_WF + NKB * WIDTH   # [128, 1024] replicated row
PK_COLS = COL_BIAS + WIDTH        # 25096
LEN_C = PK_COLS - COL_WF


def build_kernel(reps=1):
    nc = bacc.Bacc(None, target_bir_lowering=False)

    pk_d = nc.dram_tensor("pk", [128, PK_COLS], DT, kind="ExternalInput")
    out_d = nc.dram_tensor("out", [T, WIDTH], F32, kind="ExternalOutput")

    with tile.TileContext(nc) as tc:
        for rep in range(reps):
            _build_body(nc, tc, rep, pk_d, out_d)

    return nc


def _build_body(nc, tc, rep, pk_d, out_d):
    with tc.tile_pool(name=f"persist{rep}", bufs=1) as pp:
        # ---- staged input loads (3 DMAs; compute starts after A) ----
        pkA = pp.tile([128, LEN_A], DT, tag="pkA")
        nc.sync.dma_start(pkA[:], pk_d[:, 0:LEN_A])
        pkB = pp.tile([128, LEN_B], DT, tag="pkB")
        nc.sync.dma_start(pkB[:], pk_d[:, LEN_A : LEN_A + LEN_B])
        pkC = pp.tile([128, LEN_C], DT, tag="pkC")
        nc.sync.dma_start(pkC[:], pk_d[:, COL_WF:PK_COLS])

        xkv = [pkA[:, KV * i : KV * (i + 1)] for i in range(NKB)]
        wk = [pkA[:, COL_WK + HD * i : COL_WK + HD * (i + 1)] for i in range(NKB)]
        wv = [pkA[:, COL_WV + HD * i : COL_WV + HD * (i + 1)] for i in range(NKB)]
        wq = [
            pkB[:, WIDTH * i : WIDTH * (i + 1)] for i in range(NKB)
        ]  # B-local offset: COL_WQ - LEN_A == 0
        tri = pkB[:, COL_TRI - LEN_A : COL_TRI - LEN_A + 512]
        valid = pkB[:, COL_VALID - LEN_A : COL_VALID - LEN_A + NSB]
        wf = [pkC[:, WIDTH * i : WIDTH * (i + 1)] for i in range(NKB)]
        bias_bf = pkC[:, COL_BIAS - COL_WF : COL_BIAS - COL_WF + WIDTH]

        ident = pp.tile([128, 128], DT, tag="ident")
        make_identity(nc, ident[:])

        # ---- persistent intermediates ----
        qT_t = [pp.tile([128, T], DT, tag=f"qT{i}", name=f"qT{i}") for i in range(NKB)]
        ktd = pp.tile([128, KV], DT, tag="ktd")
        vaug = pp.tile([128, NSB, HD + 1], DT, tag="vaug")
        attn_t = [pp.tile([128, WIDTH], DT, tag=f"attn{i}", name=f"attn{i}") for i in range(NTB)]
        attnT_t = [pp.tile([128, T], DT, tag=f"attnT{i}", name=f"attnT{i}") for i in range(NKB)]
        bias_f = pp.tile([128, WIDTH], F32, tag="biasf")
        nc.scalar.copy(bias_f[:], bias_bf)

        # ---- phase 1: projections ----
        with (
            tc.tile_pool(name=f"psq{rep}", bufs=2, space="PSUM") as psq_pool,
            tc.tile_pool(name=f"psk{rep}", bufs=1, space="PSUM") as psk_pool,
            tc.tile_pool(name=f"psv{rep}", bufs=1, space="PSUM") as psv_pool,
        ):
            for mb in range(NKB):
                pq = psq_pool.tile([128, T], F32, tag="pq")
                for kb in range(NKB):
                    nc.tensor.matmul(
                        pq[:],
                        lhsT=wq[kb][:, 128 * mb : 128 * (mb + 1)],
                        rhs=xkv[kb][:, WIN : WIN + T],
                        start=(kb == 0),
                        stop=(kb == NKB - 1),
                    )
                nc.scalar.copy(qT_t[mb][:], pq[:])

            pk_ps = psk_pool.tile([128, KV], F32, tag="pk")
            for half in (0, 64):
                for seg0, segw in ((0, 512), (512, 256)):
                    for kb in range(NKB):
                        nc.tensor.matmul(
                            pk_ps[half : half + 64, seg0 : seg0 + segw],
                            lhsT=wk[kb],
                            rhs=xkv[kb][:, seg0 : seg0 + segw],
                            start=(kb == 0),
                            stop=(kb == NKB - 1),
                        )
            nc.vector.tensor_copy(ktd[:], pk_ps[:])

            pv = psv_pool.tile([128, NSB, HD], F32, tag="pv")
            for sb in range(NSB):
                for kb in range(NKB):
                    nc.tensor.matmul(
                        pv[:, sb, :],
                        lhsT=xkv[kb][:, 128 * sb : 128 * (sb + 1)],
                        rhs=wv[kb],
                        start=(kb == 0),
                        stop=(kb == NKB - 1),
                    )
            nc.scalar.copy(vaug[:, :, 0:HD], pv[:])
            nc.vector.tensor_copy(
                vaug[:, :, HD : HD + 1], valid.rearrange("p (s o) -> p s o", o=1)
            )

        # ---- phase 2: attention (logits computed transposed) ----
        # pl layout per head-pair, per tb: [128, 2(half), 3(j), 128]
        # j: 0 -> k3=0 (mask lo), 1 -> k3=2 (mask hi), 2 -> k3=1 (no mask)
        J2K3 = (0, 2, 1)
        with (
            tc.tile_pool(name=f"psl{rep}", bufs=2, space="PSUM") as psl_pool,
            tc.tile_pool(name=f"pso{rep}", bufs=2, space="PSUM") as pso_pool,
            tc.tile_pool(name=f"awork{rep}", bufs=3) as awork,
        ):
            for mb in range(NKB):  # head pair (2*mb, 2*mb+1)
                qh = qT_t[mb]
                po = pso_pool.tile([128, 2, NTB, 128], F32, tag="po")  # padded to 128 for PSUM bank alignment
                for tb in range(NTB):
                    # [128, 2, 4, 128]: each half starts on a PSUM bank
                    # boundary; slot j=3 is padding so no op crosses banks
                    pl = psl_pool.tile([128, 2, 4, 128], F32, tag="pl")
                    for half in (0, 1):
                        hb = 64 * half
                        for j in range(3):
                            sb = tb + J2K3[j]
                            nc.tensor.matmul(
                                pl[:, half, j, :],
                                lhsT=ktd[hb : hb + 64, 128 * sb : 128 * (sb + 1)],
                                rhs=qh[hb : hb + 64, 128 * tb : 128 * (tb + 1)],
                                start=True,
                                stop=True,
                            )
                    probsT = awork.tile([128, 2, 3, 128], DT, tag="probsT")
                    for half in (0, 1):
                        nc.scalar.activation(
                            out=probsT[:, half],
                            in_=pl[:, half, 0:3, :],
                            func=mybir.ActivationFunctionType.Exp,
                            scale=0.125,
                        )
                        nc.vector.tensor_mul(
                            probsT[:, half, 0:2, :],
                            probsT[:, half, 0:2, :],
                            tri[:, 0:256].rearrange("p (j t) -> p j t", j=2),
                        )
                    for half in (0, 1):
                        for j in range(3):
                            k3 = J2K3[j]
                            nc.tensor.matmul(
                                po[:, half, tb, 0 : HD + 1],
                                lhsT=probsT[:, half, j, :],
                                rhs=vaug[:, tb + k3, :],
                                start=(j == 0),
                                stop=(j == 2),
                            )
                recip = awork.tile([128, 2, NTB, 1], F32, tag="recip")
                nc.vector.reciprocal(recip[:], po[:, :, :, HD : HD + 1])
                for half in (0, 1):
                    h = 2 * mb + half
                    for tb in range(NTB):
                        nc.vector.tensor_scalar_mul(
                            attn_t[tb][:, 64 * h : 64 * (h + 1)],
                            po[:, half, tb, 0:HD],
                            recip[:, half, tb, :],
                        )

        # attn -> attnT for the final projection
        with (
            tc.tile_pool(name=f"psat{rep}", bufs=2, space="PSUM") as psat_pool,
        ):
            for wb in range(NKB):
                pat = psat_pool.tile([128, NTB, 128], DT, tag="pat")
                for tb in range(NTB):
                    nc.tensor.transpose(
                        pat[:, tb, :],
                        attn_t[tb][:, 128 * wb : 128 * (wb + 1)],
                        ident[:],
                    )
                nc.vector.tensor_copy(attnT_t[wb][:], pat[:])

        # ---- phase 3: final projection + bias ----
        with (
            tc.tile_pool(name=f"psf{rep}", bufs=4, space="PSUM") as psf_pool,
            tc.tile_pool(name=f"fin{rep}", bufs=3) as fin_pool,
        ):
            for tb in range(NTB):
                for nh in range(2):
                    pf = psf_pool.tile([128, 512], F32, tag="pf")
                    for wb in range(NKB):
                        nc.tensor.matmul(
                            pf[:],
                            lhsT=attnT_t[wb][:, 128 * tb : 128 * (tb + 1)],
                            rhs=wf[wb][:, 512 * nh : 512 * (nh + 1)],
                            start=(wb == 0),
                            stop=(wb == NKB - 1),
                        )
                    fo = fin_pool.tile([128, 512], F32, tag="fo")
                    nc.vector.tensor_add(
                        fo[:], pf[:], bias_f[:, 512 * nh : 512 * (nh + 1)]
                    )
                    nc.sync.dma_start(
                        out_d[128 * tb : 128 * (tb + 1), 512 * nh : 512 * (nh + 1)],
                        fo[:],
                    )


def _prep_core_inputs(x, Wq, Wk, Wv, Wf, bf, core):
    bi, ch = divmod(core, 4)
    qs = T * ch
    ks = qs - WIN
    xkvT = np.zeros((WIDTH, KV), np.float32)
    lo = max(ks, 0)
    xkvT[:, lo - ks :] = x[bi, lo : qs + T, :].T

    pk = np.zeros((128, PK_COLS), np.float32)
    for i in range(NKB):
        pk[:, COL_XKV + KV * i : COL_XKV + KV * (i + 1)] = xkvT[128 * i : 128 * (i + 1)]
        pk[:, COL_WK + HD * i : COL_WK + HD * (i + 1)] = Wk.T[128 * i : 128 * (i + 1)]
        pk[:, COL_WV + HD * i : COL_WV + HD * (i + 1)] = Wv.T[128 * i : 128 * (i + 1)]
        pk[:, COL_WQ + WIDTH * i : COL_WQ + WIDTH * (i + 1)] = Wq.T[
            128 * i : 128 * (i + 1)
        ]
        pk[:, COL_WF + WIDTH * i : COL_WF + WIDTH * (i + 1)] = Wf.T[
            128 * i : 128 * (i + 1)
        ]

    s = np.arange(128)[:, None]
    t = np.arange(128)[None, :]
    tri_lo = (s >= t).astype(np.float32)  # k3=0 block: keep s >= t
    tri_hi = (s <= t).astype(np.float32)  # k3=2 block: keep s <= t
    pk[:, COL_TRI + 0 : COL_TRI + 128] = tri_lo
    pk[:, COL_TRI + 128 : COL_TRI + 256] = tri_hi
    pk[:, COL_TRI + 256 : COL_TRI + 384] = tri_lo
    pk[:, COL_TRI + 384 : COL_TRI + 512] = tri_hi

    # kv validity per 128-block: 0 for the zero-padded halo of chunk 0
    kv_pos = ks + (np.arange(NSB)[None, :] * 128 + np.arange(128)[:, None])
    pk[:, COL_VALID : COL_VALID + NSB] = (kv_pos >= 0).astype(np.float32)

    pk[:, COL_BIAS : COL_BIAS + WIDTH] = np.broadcast_to(
        bf.astype(np.float32), (128, WIDTH)
    )
    return {"pk": pk.astype(NPDT)}


_RUN_KW = {}  # test.py can inject trace=True etc.
_LAST_RESULT = [None]


def kernel(x, segment_pos, Wq, Wk, Wv, Wf, bf):
    x = np.asarray(x, np.float32)
    Wq = np.asarray(Wq, np.float32)
    Wk = np.asarray(Wk, np.float32)
    Wv = np.asarray(Wv, np.float32)
    Wf = np.asarray(Wf, np.float32)
    bf = np.asarray(bf, np.float32)

    nc = build_kernel()
    nc.finalize()
    in_maps = [_prep_core_inputs(x, Wq, Wk, Wv, Wf, bf, c) for c in range(8)]
    res = run_bass_kernel_spmd(nc, in_maps, core_ids=list(range(8)), **_RUN_KW)
    _LAST_RESULT[0] = res

    b, t = x.shape[0], x.shape[1]
    out = np.empty((b, t, WIDTH), np.float32)
    for c in range(8):
        bi, ch = divmod(c, 4)
        out[bi, T * ch : T * (ch + 1)] = res.results[c]["out"]
    return out


# revision 10
# speedup vs baseline: 56.0813x; 55.0706x over previous
"""Sliding-window MQA attention block on Trainium2 (single NeuronCore).

The full problem (batch 2 x 2048 tokens) is processed as 8 sequential
chunk-bodies of 512 query tokens on ONE core. Measured through this
container's axon-tunneled PJRT stack, per-execution dispatch cost scales
with the number of participating devices (~1.4 ms at 1 device vs ~6.4 ms
at 8) while the whole problem's device compute (~0.5 ms) fits inside a
single device's dispatch shadow -- so one core minimizes end-to-end
latency even though 8 cores are available. The chunk loop is
instruction-level parallel: the Tile scheduler overlaps chunk i+1's
DMA/projections with chunk i's attention/output.

Each chunk-body sees its 512 query tokens plus a 256-token K/V halo
(768 KV tokens, zero-padded in front for chunk 0 of each batch).
Shared weights are DMA'd to SBUF once; only the per-chunk activations
(x^T slice + kv-validity) stream per body.

Device algorithm per chunk, logits computed TRANSPOSED ([s, t]) so no
PE transposes of probs are needed:
  qT[1024, 512]  = WqT.T @ xqT            (per 128-row blocks; [hd, t])
  ktd[128, 768]  = K^T duplicated into both partition halves (MQA shared)
  vaug[128,6,65] = V with a validity column (gives softmax denominators
                   AND zeroes out the padded kv positions of chunk 0)
  per head-pair, per 128-query block tb (window = 3 kv blocks, diagonal):
    logitsT[s,t] blocks via matmul(lhsT=ktd[hd, s-blk], rhs=qT[hd, t-blk])
    probsT = exp(0.125 * logitsT)
    two triangular 128x128 masks (k3=0 lower, k3=2 upper) in one strided
    vector multiply; the middle diagonal needs no mask
    po[t, 65] += probsT_blk.T @ vaug_blk  (PE, accumulate 3 diagonals)
    attn[t, 64h:64h+64] = po[:, :64] * (1 / po[:, 64])
  attnT via PE transpose; final[512, 1024] = attnT.T @ WfT + bias
"""

import math
import os
import sys

import numpy as np

for _p in ("/opt/trn_rl_repo",):
    if _p not in sys.path and os.path.isdir(_p):
        sys.path.insert(0, _p)

import ml_dtypes

import concourse.bass as bass
import concourse.mybir as mybir
import concourse.tile as tile
from concourse import bacc
from concourse.bass_utils import run_bass_kernel_spmd
from concourse.masks import make_identity

WIDTH = 1024
H = 16
HD = 64
WIN = 256
T = 512          # query tokens per chunk
KV = 768         # kv tokens per chunk (256 halo + 512)
NKB = WIDTH // 128
NTB = T // 128
NSB = KV // 128
CHUNKS = 8       # 2 batches x 4 query chunks
F32 = mybir.dt.float32
DT = mybir.dt.bfloat16
NPDT = ml_dtypes.bfloat16

# packed input layout (bf16 columns): shared weight block, then one
# activation block per chunk
S_WK = 0                          # 8 x 64
S_WV = S_WK + NKB * HD            # 8 x 64
S_WQ = S_WV + NKB * HD            # 8 x 1024
S_TRI = S_WQ + NKB * WIDTH        # [128, 512]: lo|hi|lo|hi
S_BIAS = S_TRI + 512              # [128, 1024] replicated row
LEN_S1 = S_BIAS + WIDTH           # staging tile S1 (wk|wv|wq|tri|bias)
S_WF = LEN_S1                     # 8 x 1024
LEN_S = S_WF + NKB * WIDTH        # shared block total (18944)
CK_XKV = 0                        # 8 x 768 (within a chunk block)
CK_VALID = CK_XKV + NKB * KV      # [128, 8] (6 used)
LEN_CK = CK_VALID + 8             # per-chunk block total (6152)
PK_COLS = LEN_S + CHUNKS * LEN_CK


def build_kernel(reps=1):
    nc = bacc.Bacc(None, target_bir_lowering=False)

    pk_d = nc.dram_tensor("pk", [128, PK_COLS], DT, kind="ExternalInput")
    out_d = nc.dram_tensor("out", [CHUNKS * T, WIDTH], F32, kind="ExternalOutput")

    with tile.TileContext(nc) as tc:
        for rep in range(reps):
            with tc.tile_pool(name=f"shared{rep}", bufs=1) as sp:
                pkS1 = sp.tile([128, LEN_S1], DT, tag="pkS1")
                nc.sync.dma_start(pkS1[:], pk_d[:, 0:LEN_S1])
                pkS2 = sp.tile([128, LEN_S - LEN_S1], DT, tag="pkS2")
                nc.sync.dma_start(pkS2[:], pk_d[:, LEN_S1:LEN_S])

                shared = {
                    "wk": [pkS1[:, S_WK + HD * i : S_WK + HD * (i + 1)] for i in range(NKB)],
                    "wv": [pkS1[:, S_WV + HD * i : S_WV + HD * (i + 1)] for i in range(NKB)],
                    "wq": [pkS1[:, S_WQ + WIDTH * i : S_WQ + WIDTH * (i + 1)] for i in range(NKB)],
                    "tri": pkS1[:, S_TRI : S_TRI + 512],
                    "bias": pkS1[:, S_BIAS : S_BIAS + WIDTH],
                    "wf": [pkS2[:, WIDTH * i : WIDTH * (i + 1)] for i in range(NKB)],
                }
                ident = sp.tile([128, 128], DT, tag="ident")
                make_identity(nc, ident[:])
                bias_f = sp.tile([128, WIDTH], F32, tag="biasf")
                nc.scalar.copy(bias_f[:], shared["bias"])
                shared["bias_f"] = bias_f
                shared["ident"] = ident

                for c in range(CHUNKS):
                    _build_body(nc, tc, f"{rep}c{c}", pk_d, out_d, shared, c)

    return nc


def _build_body(nc, tc, rep, pk_d, out_d, shared, c):
    base = LEN_S + c * LEN_CK
    row0 = c * T
    wq, wk, wv, wf = shared["wq"], shared["wk"], shared["wv"], shared["wf"]
    tri, bias_f, ident = shared["tri"], shared["bias_f"], shared["ident"]

    with tc.tile_pool(name=f"persist{rep}", bufs=1) as pp:
        pkX = pp.tile([128, LEN_CK], DT, tag="pkX")
        nc.sync.dma_start(pkX[:], pk_d[:, base : base + LEN_CK])
        xkv = [pkX[:, KV * i : KV * (i + 1)] for i in range(NKB)]
        valid = pkX[:, CK_VALID : CK_VALID + NSB]

        # ---- persistent intermediates ----
        qT_t = [pp.tile([128, T], DT, tag=f"qT{i}", name=f"qT{i}") for i in range(NKB)]
        ktd = pp.tile([128, KV], DT, tag="ktd")
        vaug = pp.tile([128, NSB, HD + 1], DT, tag="vaug")
        attn_t = [
            pp.tile([128, WIDTH], DT, tag=f"attn{i}", name=f"attn{i}")
            for i in range(NTB)
        ]
        attnT_t = [
            pp.tile([128, T], DT, tag=f"attnT{i}", name=f"attnT{i}")
            for i in range(NKB)
        ]

        # ---- phase 1: projections ----
        with (
            tc.tile_pool(name=f"psq{rep}", bufs=2, space="PSUM") as psq_pool,
            tc.tile_pool(name=f"psk{rep}", bufs=1, space="PSUM") as psk_pool,
            tc.tile_pool(name=f"psv{rep}", bufs=1, space="PSUM") as psv_pool,
        ):
            for mb in range(NKB):
                pq = psq_pool.tile([128, T], F32, tag="pq")
                for kb in range(NKB):
                    nc.tensor.matmul(
                        pq[:],
                        lhsT=wq[kb][:, 128 * mb : 128 * (mb + 1)],
                        rhs=xkv[kb][:, WIN : WIN + T],
                        start=(kb == 0),
                        stop=(kb == NKB - 1),
                    )
                nc.scalar.copy(qT_t[mb][:], pq[:])

            pk_ps = psk_pool.tile([128, KV], F32, tag="pk")
            for half in (0, 64):
                for seg0, segw in ((0, 512), (512, 256)):
                    for kb in range(NKB):
                        nc.tensor.matmul(
                            pk_ps[half : half + 64, seg0 : seg0 + segw],
                            lhsT=wk[kb],
                            rhs=xkv[kb][:, seg0 : seg0 + segw],
                            start=(kb == 0),
                            stop=(kb == NKB - 1),
                        )
            nc.vector.tensor_copy(ktd[:], pk_ps[:])

            pv = psv_pool.tile([128, NSB, HD], F32, tag="pv")
            for sb in range(NSB):
                for kb in range(NKB):
                    nc.tensor.matmul(
                        pv[:, sb, :],
                        lhsT=xkv[kb][:, 128 * sb : 128 * (sb + 1)],
                        rhs=wv[kb],
                        start=(kb == 0),
                        stop=(kb == NKB - 1),
                    )
            nc.scalar.copy(vaug[:, :, 0:HD], pv[:])
            nc.vector.tensor_copy(
                vaug[:, :, HD : HD + 1], valid.rearrange("p (s o) -> p s o", o=1)
            )

        # ---- phase 2: attention (logits computed transposed) ----
        # pl layout per head-pair, per tb: [128, 2(half), 4(j; 3 used), 128]
        # j: 0 -> k3=0 (mask lo), 1 -> k3=2 (mask hi), 2 -> k3=1 (no mask)
        J2K3 = (0, 2, 1)
        with (
            tc.tile_pool(name=f"psl{rep}", bufs=2, space="PSUM") as psl_pool,
            tc.tile_pool(name=f"pso{rep}", bufs=2, space="PSUM") as pso_pool,
            tc.tile_pool(name=f"awork{rep}", bufs=3) as awork,
        ):
            for mb in range(NKB):  # head pair (2*mb, 2*mb+1)
                qh = qT_t[mb]
                # padded to 128 so no PE write crosses a PSUM bank
                po = pso_pool.tile([128, 2, NTB, 128], F32, tag="po")
                for tb in range(NTB):
                    # each half starts on a PSUM bank boundary; slot j=3
                    # is padding so no op crosses banks
                    pl = psl_pool.tile([128, 2, 4, 128], F32, tag="pl")
                    for half in (0, 1):
                        hb = 64 * half
                        for j in range(3):
                            sb = tb + J2K3[j]
                            nc.tensor.matmul(
                                pl[:, half, j, :],
                                lhsT=ktd[hb : hb + 64, 128 * sb : 128 * (sb + 1)],
                                rhs=qh[hb : hb + 64, 128 * tb : 128 * (tb + 1)],
                                start=True,
                                stop=True,
                            )
                    probsT = awork.tile([128, 2, 3, 128], DT, tag="probsT")
                    for half in (0, 1):
                        nc.scalar.activation(
                            out=probsT[:, half],
                            in_=pl[:, half, 0:3, :],
                            func=mybir.ActivationFunctionType.Exp,
                            scale=0.125,
                        )
                        nc.vector.tensor_mul(
                            probsT[:, half, 0:2, :],
                            probsT[:, half, 0:2, :],
                            tri[:, 0:256].rearrange("p (j t) -> p j t", j=2),
                        )
                    for half in (0, 1):
                        for j in range(3):
                            k3 = J2K3[j]
                            nc.tensor.matmul(
                                po[:, half, tb, 0 : HD + 1],
                                lhsT=probsT[:, half, j, :],
                                rhs=vaug[:, tb + k3, :],
                                start=(j == 0),
                                stop=(j == 2),
                            )
                recip = awork.tile([128, 2, NTB, 1], F32, tag="recip")
                nc.vector.reciprocal(recip[:], po[:, :, :, HD : HD + 1])
                for half in (0, 1):
                    h = 2 * mb + half
                    for tb in range(NTB):
                        nc.vector.tensor_scalar_mul(
                            attn_t[tb][:, 64 * h : 64 * (h + 1)],
                            po[:, half, tb, 0:HD],
                            recip[:, half, tb, :],
                        )

        # attn -> attnT for the final projection
        with (
            tc.tile_pool(name=f"psat{rep}", bufs=2, space="PSUM") as psat_pool,
        ):
            for wb in range(NKB):
                pat = psat_pool.tile([128, NTB, 128], DT, tag="pat")
                for tb in range(NTB):
                    nc.tensor.transpose(
                        pat[:, tb, :],
                        attn_t[tb][:, 128 * wb : 128 * (wb + 1)],
                        ident[:],
                    )
                nc.vector.tensor_copy(attnT_t[wb][:], pat[:])

        # ---- phase 3: final projection + bias ----
        with (
            tc.tile_pool(name=f"psf{rep}", bufs=4, space="PSUM") as psf_pool,
            tc.tile_pool(name=f"fin{rep}", bufs=3) as fin_pool,
        ):
            for tb in range(NTB):
                for nh in range(2):
                    pf = psf_pool.tile([128, 512], F32, tag="pf")
                    for wb in range(NKB):
                        nc.tensor.matmul(
                            pf[:],
                            lhsT=attnT_t[wb][:, 128 * tb : 128 * (tb + 1)],
                            rhs=wf[wb][:, 512 * nh : 512 * (nh + 1)],
                            start=(wb == 0),
                            stop=(wb == NKB - 1),
                        )
                    fo = fin_pool.tile([128, 512], F32, tag="fo")
                    nc.vector.tensor_add(
                        fo[:], pf[:], bias_f[:, 512 * nh : 512 * (nh + 1)]
                    )
                    nc.sync.dma_start(
                        out_d[
                            row0 + 128 * tb : row0 + 128 * (tb + 1),
                            512 * nh : 512 * (nh + 1),
                        ],
                        fo[:],
                    )


def prep_inputs(x, Wq, Wk, Wv, Wf, bf):
    """Pack everything into one [128, PK_COLS] bf16 array."""
    pk = np.zeros((128, PK_COLS), np.float32)
    for i in range(NKB):
        pk[:, S_WK + HD * i : S_WK + HD * (i + 1)] = Wk.T[128 * i : 128 * (i + 1)]
        pk[:, S_WV + HD * i : S_WV + HD * (i + 1)] = Wv.T[128 * i : 128 * (i + 1)]
        pk[:, S_WQ + WIDTH * i : S_WQ + WIDTH * (i + 1)] = Wq.T[128 * i : 128 * (i + 1)]
        pk[:, S_WF + WIDTH * i : S_WF + WIDTH * (i + 1)] = Wf.T[128 * i : 128 * (i + 1)]

    s = np.arange(128)[:, None]
    t = np.arange(128)[None, :]
    tri_lo = (s >= t).astype(np.float32)  # k3=0 block: keep s >= t
    tri_hi = (s <= t).astype(np.float32)  # k3=2 block: keep s <= t
    pk[:, S_TRI + 0 : S_TRI + 128] = tri_lo
    pk[:, S_TRI + 128 : S_TRI + 256] = tri_hi
    pk[:, S_TRI + 256 : S_TRI + 384] = tri_lo
    pk[:, S_TRI + 384 : S_TRI + 512] = tri_hi
    pk[:, S_BIAS : S_BIAS + WIDTH] = np.broadcast_to(bf.astype(np.float32), (128, WIDTH))

    for c in range(CHUNKS):
        bi, ch = divmod(c, 4)
        qs = T * ch
        ks = qs - WIN
        base = LEN_S + c * LEN_CK
        xkvT = np.zeros((WIDTH, KV), np.float32)
        lo = max(ks, 0)
        xkvT[:, lo - ks :] = x[bi, lo : qs + T, :].T
        for i in range(NKB):
            pk[:, base + CK_XKV + KV * i : base + CK_XKV + KV * (i + 1)] = xkvT[
                128 * i : 128 * (i + 1)
            ]
        kv_pos = ks + (np.arange(NSB)[None, :] * 128 + np.arange(128)[:, None])
        pk[:, base + CK_VALID : base + CK_VALID + NSB] = (kv_pos >= 0).astype(
            np.float32
        )
    return {"pk": pk.astype(NPDT)}


_RUN_KW = {}  # test.py can inject trace=True etc.
_LAST_RESULT = [None]


def kernel(x, segment_pos, Wq, Wk, Wv, Wf, bf):
    x = np.asarray(x, np.float32)
    Wq = np.asarray(Wq, np.float32)
    Wk = np.asarray(Wk, np.float32)
    Wv = np.asarray(Wv, np.float32)
    Wf = np.asarray(Wf, np.float32)
    bf = np.asarray(bf, np.float32)

    nc = build_kernel()
    nc.finalize()
    in_maps = [prep_inputs(x, Wq, Wk, Wv, Wf, bf)]
    res = run_bass_kernel_spmd(nc, in_maps, core_ids=[0], **_RUN_KW)
    _LAST_RESULT[0] = res

    b, t = x.shape[0], x.shape[1]
    return np.ascontiguousarray(
        res.results[0]["out"].reshape(b, t, WIDTH).astype(np.float32)
    )
